# revision 7
# baseline (speedup 1.0000x reference)
"""Trainium2 Bass kernel for the two-stage DAN/MoVe attention module.

Computation (per batch b):
  Stage 1:  S  = skT.T @ q1 / sqrt(C);  P  = softmax_k(S);   newV^T = P.T-contracted with svT
            (computed as newVT[q, vc] = sum_k P[k, q] * svT[k, vc])
  Stage 2:  S2 = mK.T @ qq / sqrt(C);   P2 = softmax_k2(S2); out^T[q2, vc] = sum_k2 P2[k2, q2] * nvtn[k2, vc]

Sharding: 8 cores = 2 batches x 4 lanes. Stage 1 splits the 1600 query
columns 4 ways (400 each); stage 2 splits the 14400 frame-query columns
4 ways (3712-wide windows, 3600 owned). Two SPMD launches; the host
passes stage-1 results (unnormalized newVT + column sums) into stage 2,
where normalization happens on-device via per-partition reciprocal scales.

Matmuls run in float32r (single-pass fp32 PE mode, ~1.5e-4 rel err,
4x faster than fp32). Softmax skips the max-subtraction (scores are
~N(0,1); exp cannot overflow) so only exp + column sums are needed;
column sums come from ones-matmuls on the tensor engine.
"""

import math

import numpy as np

import concourse.bass as bass
import concourse.tile as tile
from concourse import bacc, mybir
from concourse.bass_utils import run_bass_kernel_spmd

F32 = mybir.dt.float32
F32R = mybir.dt.float32r
EXP = mybir.ActivationFunctionType.Exp

B, FRAME, SFRAME, C, VC, H, W = 2, 9, 15, 128, 512, 40, 40
HW = H * W                      # 1600
MID = FRAME // 2                # 4
WK = SFRAME * HW                # 24000 support keys
NKT = (WK + 127) // 128         # 188 key tiles (last = 64 rows)
Q2 = FRAME * HW                 # 14400 stage-2 query columns per batch
NK2T = (HW + 127) // 128        # 13 stage-2 key tiles (last = 64 rows)

L1_COLS = HW // 4               # 400 owned stage-1 columns per lane
L1_SUBS = [(0, 128), (128, 128), (256, 128), (384, 16)]
L2_WIN = 3712                   # 29 * 128, per-lane stage-2 window
L2_OWN = Q2 // 4                # 3600 owned columns
L2_CHUNKS = [512] * 7 + [128]
INV_SQRT_C = 1.0 / math.sqrt(C)

_cache = {}


def _build_stage1():
    nc = bacc.Bacc("TRN2", target_bir_lowering=False, debug=False, num_devices=8)
    skt = nc.dram_tensor("skt", [C, WK], F32R, kind="ExternalInput").ap()
    svt = nc.dram_tensor("svt", [WK, VC], F32R, kind="ExternalInput").ap()
    q1 = nc.dram_tensor("q1", [C, L1_COLS], F32R, kind="ExternalInput").ap()
    ones = nc.dram_tensor("ones", [128, 2], F32R, kind="ExternalInput").ap()
    mun = nc.dram_tensor("mun", [L1_COLS, VC], F32, kind="ExternalOutput").ap()
    csum = nc.dram_tensor("csum", [2, L1_COLS], F32, kind="ExternalOutput").ap()

    with tile.TileContext(nc) as tc:
        with (
            tc.tile_pool(name="const", bufs=1) as cpool,
            tc.tile_pool(name="skt", bufs=1) as skpool,
            tc.tile_pool(name="svt", bufs=4) as svpool,
            tc.tile_pool(name="p", bufs=3) as ppool,
            tc.tile_pool(name="out", bufs=5) as opool,
            tc.tile_pool(name="ps_s", bufs=2, space="PSUM") as ps_s,
            tc.tile_pool(name="ps_m", bufs=1, space="PSUM") as ps_m,
            tc.tile_pool(name="ps_c", bufs=1, space="PSUM") as ps_c,
        ):
            q1_t = cpool.tile([C, L1_COLS], F32R)
            nc.sync.dma_start(q1_t[:], q1[:])
            ones_t = cpool.tile([128, 2], F32R)
            nc.sync.dma_start(ones_t[:], ones[:])

            # whole skT resident, loaded in chunks so early tiles unblock fast
            skt_t = skpool.tile([C, WK], F32R)
            for o in range(0, WK, 4000):
                nc.sync.dma_start(skt_t[:, o:o + 4000], skt[:, o:o + 4000])

            m_ps = [ps_m.tile([128, VC], F32, name=f"m_ps{s}", tag=f"m_ps{s}")
                    for s in range(len(L1_SUBS))]
            c_ps = ps_c.tile([2, L1_COLS], F32)

            for kt in range(NKT):
                kk = min(128, WK - kt * 128)
                r0 = kt * 128
                sv_t = svpool.tile([128, VC], F32R)
                nc.sync.dma_start(sv_t[:kk, :], svt[r0:r0 + kk, :])
                s_ps = ps_s.tile([128, L1_COLS], F32)
                nc.tensor.matmul(s_ps[:kk, :], skt_t[:, r0:r0 + kk], q1_t[:],
                                 start=True, stop=True)
                p_t = ppool.tile([128, L1_COLS], F32R)
                nc.scalar.activation(p_t[:kk, :], s_ps[:kk, :], EXP,
                                     scale=INV_SQRT_C)
                nc.tensor.matmul(c_ps[:], ones_t[:kk, :], p_t[:kk, :],
                                 start=(kt == 0), stop=(kt == NKT - 1))
                for s, (o, w) in enumerate(L1_SUBS):
                    nc.tensor.matmul(m_ps[s][:w, :], p_t[:kk, o:o + w],
                                     sv_t[:kk, :],
                                     start=(kt == 0), stop=(kt == NKT - 1))

            for s, (o, w) in enumerate(L1_SUBS):
                m_sb = opool.tile([128, VC], F32)
                nc.vector.tensor_copy(m_sb[:w, :], m_ps[s][:w, :])
                nc.sync.dma_start(mun[o:o + w, :], m_sb[:w, :])
            c_sb = opool.tile([2, L1_COLS], F32)
            nc.vector.tensor_copy(c_sb[:], c_ps[:])
            nc.sync.dma_start(csum[:], c_sb[:])
    nc.compile()
    return nc


def _build_stage2():
    nc = bacc.Bacc("TRN2", target_bir_lowering=False, debug=False, num_devices=8)
    mk = nc.dram_tensor("mk", [C, HW], F32R, kind="ExternalInput").ap()
    qq = nc.dram_tensor("qq", [C, L2_WIN], F32R, kind="ExternalInput").ap()
    nvt = nc.dram_tensor("nvt", [HW, VC], F32R, kind="ExternalInput").ap()
    cs1 = nc.dram_tensor("cs1", [HW, 1], F32, kind="ExternalInput").ap()
    ones = nc.dram_tensor("ones", [128, 2], F32R, kind="ExternalInput").ap()
    out = nc.dram_tensor("out", [L2_WIN, VC], F32, kind="ExternalOutput").ap()

    with tile.TileContext(nc) as tc:
        with (
            tc.tile_pool(name="const", bufs=1) as cpool,
            tc.tile_pool(name="nvt", bufs=1) as nvpool,
            tc.tile_pool(name="small", bufs=4) as smpool,
            tc.tile_pool(name="p2", bufs=26) as p2pool,
            tc.tile_pool(name="ob", bufs=3) as obpool,
            tc.tile_pool(name="ps_s", bufs=2, space="PSUM") as ps_s,
            tc.tile_pool(name="ps_o", bufs=2, space="PSUM") as ps_o,
            tc.tile_pool(name="ps_c", bufs=2, space="PSUM") as ps_c,
        ):
            mk_t = cpool.tile([C, HW], F32R)
            nc.sync.dma_start(mk_t[:], mk[:])
            ones_t = cpool.tile([128, 2], F32R)
            nc.sync.dma_start(ones_t[:], ones[:])
            qq_t = cpool.tile([C, L2_WIN], F32R)
            for o in range(0, L2_WIN, 1856):
                nc.sync.dma_start(qq_t[:, o:o + 1856], qq[:, o:o + 1856])

            # load newVT tiles, normalize by stage-1 column sums (per-partition)
            nvtn = []
            for t in range(NK2T):
                kk = min(128, HW - t * 128)
                r0 = t * 128
                raw = smpool.tile([128, VC], F32R, tag="nvraw")
                nc.sync.dma_start(raw[:kk, :], nvt[r0:r0 + kk, :])
                cs_t = smpool.tile([128, 1], F32, tag="cs")
                nc.sync.dma_start(cs_t[:kk, :], cs1[r0:r0 + kk, :])
                rc_t = smpool.tile([128, 1], F32, tag="rc")
                nc.vector.reciprocal(rc_t[:kk, :], cs_t[:kk, :])
                nrm = nvpool.tile([128, VC], F32R, tag=f"nvtn{t}")
                nc.vector.tensor_scalar_mul(nrm[:kk, :], raw[:kk, :], rc_t[:kk, 0:1])
                nvtn.append(nrm)

            col = 0
            for chunk in L2_CHUNKS:
                p2 = []
                for t in range(NK2T):
                    kk = min(128, HW - t * 128)
                    s_ps = ps_s.tile([128, 512], F32)
                    nc.tensor.matmul(s_ps[:kk, :chunk],
                                     mk_t[:, t * 128:t * 128 + kk],
                                     qq_t[:, col:col + chunk],
                                     start=True, stop=True)
                    p_t = p2pool.tile([128, 512], F32R, tag="p2")
                    nc.scalar.activation(p_t[:kk, :chunk], s_ps[:kk, :chunk],
                                         EXP, scale=INV_SQRT_C)
                    p2.append(p_t)
                for sub in range(chunk // 128):
                    so = sub * 128
                    o_ps = ps_o.tile([128, VC], F32)
                    c_ps = ps_c.tile([128, 2], F32)
                    for t in range(NK2T):
                        kk = min(128, HW - t * 128)
                        nc.tensor.matmul(o_ps[:], p2[t][:kk, so:so + 128],
                                         nvtn[t][:kk, :],
                                         start=(t == 0), stop=(t == NK2T - 1))
                        nc.tensor.matmul(c_ps[:], p2[t][:kk, so:so + 128],
                                         ones_t[:kk, :],
                                         start=(t == 0), stop=(t == NK2T - 1))
                    rc = smpool.tile([128, 1], F32, tag="rc2")
                    nc.vector.reciprocal(rc[:], c_ps[:, 0:1])
                    ob = obpool.tile([128, VC], F32)
                    nc.vector.tensor_scalar_mul(ob[:], o_ps[:], rc[:, 0:1])
                    nc.sync.dma_start(out[col + so:col + so + 128, :], ob[:])
                col += chunk
    nc.compile()
    return nc


def kernel(query_q, query_k, support_k, support_v):
    query_q = np.ascontiguousarray(query_q, dtype=np.float32)
    query_k = np.ascontiguousarray(query_k, dtype=np.float32)
    support_k = np.ascontiguousarray(support_k, dtype=np.float32)
    support_v = np.ascontiguousarray(support_v, dtype=np.float32)

    if "l1" not in _cache:
        _cache["l1"] = _build_stage1()
    if "l2" not in _cache:
        _cache["l2"] = _build_stage2()

    ones = np.ones((128, 2), np.float32)

    # ---- host layout prep ----
    # skT[b]: [C, SF*HW], svT[b]: [SF*HW, VC], q1[b]: [C, HW]
    skt = support_k.transpose(0, 2, 1, 3, 4).reshape(B, C, WK)
    svt = support_v.transpose(0, 1, 3, 4, 2).reshape(B, WK, VC)
    q1 = query_q[:, MID].reshape(B, C, HW)
    l1_maps = []
    for core in range(8):
        b, lane = divmod(core, 4)
        l1_maps.append({
            "skt": skt[b],
            "svt": svt[b],
            "q1": np.ascontiguousarray(q1[b][:, lane * L1_COLS:(lane + 1) * L1_COLS]),
            "ones": ones,
        })
    res1 = run_bass_kernel_spmd(_cache["l1"], l1_maps, list(range(8)))
    r1 = res1.results

    nvt = np.empty((B, HW, VC), np.float32)
    cs1 = np.empty((B, HW, 1), np.float32)
    for core in range(8):
        b, lane = divmod(core, 4)
        sl = slice(lane * L1_COLS, (lane + 1) * L1_COLS)
        nvt[b][sl] = r1[core]["mun"]
        cs1[b][sl, 0] = r1[core]["csum"][0]

    # ---- stage 2 ----
    mk = query_k[:, MID].reshape(B, C, HW)
    qq = query_q.transpose(0, 2, 1, 3, 4).reshape(B, C, Q2)
    wins = [0, L2_OWN, 2 * L2_OWN, Q2 - L2_WIN]
    l2_maps = []
    for core in range(8):
        b, lane = divmod(core, 4)
        w = wins[lane]
        l2_maps.append({
            "mk": mk[b],
            "qq": np.ascontiguousarray(qq[b][:, w:w + L2_WIN]),
            "nvt": nvt[b],
            "cs1": cs1[b],
            "ones": ones,
        })
    res2 = run_bass_kernel_spmd(_cache["l2"], l2_maps, list(range(8)))
    r2 = res2.results
    _cache["last_exec_ns"] = [res1.exec_time_ns, res2.exec_time_ns]

    outT = np.empty((B, Q2, VC), np.float32)
    for core in range(8):
        b, lane = divmod(core, 4)
        w = wins[lane]
        lo = lane * L2_OWN - w
        outT[b][lane * L2_OWN:(lane + 1) * L2_OWN] = r2[core]["out"][lo:lo + L2_OWN]

    # outT[b][q2, vc], q2 = f*HW + h*W + w  ->  [B, F, VC, H, W]
    return np.ascontiguousarray(
        outT.reshape(B, FRAME, H, W, VC).transpose(0, 1, 4, 2, 3))
